# revision 8
# baseline (speedup 1.0000x reference)
"""Sliding-window causal self-attention (WINDOW=256) on 8 trn2 NeuronCores.

Sharding: 8 cores = 4 batch items x 2 sequence halves (1024 queries each).
Each core receives x pre-transposed with a 256-row key/value halo and
computes its output chunk fully independently (no collectives); the host
concatenates.

On-core dataflow (transpose-free, "key-major" attention):
  x_T [C=768, 1280]  --PE-->  Q_T [768, 1024], K_T [768, 1280]  (head dim on
  partitions), V [1280, 768] (+ a 64-wide ones block per head for softmax
  denominators).
  Per head, per 128-key tile: S_T [128k, 384q] = K_T.T @ Q_T (K=64 matmul),
  E = exp(S/8) via ScalarE (scale fused), band+validity mask multiply (DVE),
  then AV matmuls accumulate Y (rows 0-63) and the replicated denominator
  (rows 64-127) into PSUM per 128-query block.  Normalize with
  reciprocal_approx_fast, project with W_proj (bias via K=1 ones-row matmul).
"""

import numpy as np
import ml_dtypes

import concourse.bass as bass
import concourse.bacc as bacc
import concourse.mybir as mybir
from concourse.tile import TileContext
from concourse.bass_utils import run_bass_kernel_spmd

F32 = mybir.dt.float32
BF16 = mybir.dt.bfloat16
AF = mybir.ActivationFunctionType
OP = mybir.AluOpType

N_HEAD = 12
WINDOW = 256
B, T, C = 4, 2048, 768
HD = C // N_HEAD              # 64
TQ = 1024                     # queries per core
HALO = 256
ROWS = TQ + HALO              # 1280 rows of k/v per core
NCT = C // 128                # 6 contraction tiles
NKT = ROWS // 128             # 10 key tiles
WIN = 384                     # q-window width per key tile
SCALE = 1.0 / float(np.sqrt(HD))

# q-window start per key tile (compile-time, same on every core)
QS = [min(max(128 * (kt - 2), 0), TQ - WIN) for kt in range(NKT)]


def _build_nc() -> bass.Bass:
    nc = bacc.Bacc()

    x_t = nc.dram_tensor("x_t", [C, ROWS], BF16, kind="ExternalInput")
    wq = nc.dram_tensor("wq", [C, C], BF16, kind="ExternalInput")
    wk = nc.dram_tensor("wk", [C, C], BF16, kind="ExternalInput")
    wv = nc.dram_tensor("wv", [C, C], BF16, kind="ExternalInput")
    wp = nc.dram_tensor("wp", [C, C], BF16, kind="ExternalInput")
    bq = nc.dram_tensor("bq", [128, NCT], F32, kind="ExternalInput")
    bk = nc.dram_tensor("bk", [128, NCT], F32, kind="ExternalInput")
    bv = nc.dram_tensor("bv", [1, C], BF16, kind="ExternalInput")
    bp = nc.dram_tensor("bp", [1, C], BF16, kind="ExternalInput")
    mask = nc.dram_tensor("mask", [128, NKT * WIN], BF16, kind="ExternalInput")
    out = nc.dram_tensor("out", [TQ, C], F32, kind="ExternalOutput")

    with TileContext(nc) as tc:
        with (
            tc.tile_pool(name="persist", bufs=1) as pp,
            tc.tile_pool(name="work", bufs=3) as wk_pool,
            tc.tile_pool(name="et", bufs=2) as et_pool,
            tc.tile_pool(name="psA", bufs=3, space="PSUM") as psA,
            tc.tile_pool(name="psY", bufs=4, space="PSUM") as psY,
        ):
            # ---- persistent SBUF tensors ----
            xt_sb = pp.tile([128, NCT, ROWS], BF16)
            wq_sb = pp.tile([128, NCT, C], BF16)
            wk_sb = pp.tile([128, NCT, C], BF16)
            wv_sb = pp.tile([128, NCT, C], BF16)
            wp_sb = pp.tile([128, NCT, C], BF16)
            bq_sb = pp.tile([128, NCT], F32)
            bk_sb = pp.tile([128, NCT], F32)
            bv_sb = pp.tile([1, C], BF16)
            bp_sb = pp.tile([1, C], BF16)
            mask_sb = pp.tile([128, NKT, WIN], BF16)
            ones_sb = pp.tile([1, 128], BF16)

            qt_sb = pp.tile([128, NCT, TQ], BF16)     # Q_T: head dims on partitions
            kt_sb = pp.tile([128, NCT, ROWS], BF16)   # K_T
            v_sb = pp.tile([128, NKT, N_HEAD, 128], BF16)  # [V(64) | ones(64)]
            yn_sb = pp.tile([128, NCT, TQ], BF16)     # normalized Y_T

            nc.sync.dma_start(xt_sb[:], x_t.rearrange("(t p) n -> p t n", p=128))
            nc.sync.dma_start(wk_sb[:], wk.rearrange("(t p) n -> p t n", p=128))
            nc.sync.dma_start(wq_sb[:], wq.rearrange("(t p) n -> p t n", p=128))
            nc.sync.dma_start(wv_sb[:], wv.rearrange("(t p) n -> p t n", p=128))
            nc.sync.dma_start(wp_sb[:], wp.rearrange("(t p) n -> p t n", p=128))
            nc.sync.dma_start(bq_sb[:], bq[:])
            nc.sync.dma_start(bk_sb[:], bk[:])
            nc.sync.dma_start(bv_sb[:], bv[:])
            nc.sync.dma_start(bp_sb[:], bp[:])
            nc.sync.dma_start(mask_sb[:], mask.rearrange("p (k w) -> p k w", w=WIN))
            nc.vector.memset(ones_sb[:], 1.0)
            nc.vector.memset(v_sb[:, :, :, HD:128], 1.0)

            # ---- phase 2: projections ----
            # K_T / Q_T: out[outdim_tile, seq] = W.T @ x_T
            def proj_T(w_sb, b_sb, dst, xcol0, dcol0, ncols):
                for m in range(NCT):
                    ps = psA.tile([128, 512], F32, tag="mm")
                    for c in range(NCT):
                        nc.tensor.matmul(
                            ps[:, :ncols],
                            w_sb[:, c, m * 128:(m + 1) * 128],
                            xt_sb[:, c, xcol0:xcol0 + ncols],
                            start=(c == 0),
                            stop=(c == NCT - 1),
                        )
                    nc.scalar.activation(
                        dst[:, m, dcol0:dcol0 + ncols],
                        ps[:, :ncols],
                        AF.Identity,
                        bias=b_sb[:, m:m + 1],
                    )

            for col0, ncols in ((0, 512), (512, 512), (1024, 256)):
                proj_T(wk_sb, bk_sb, kt_sb, col0, col0, ncols)
            for col0 in (0, 512):
                proj_T(wq_sb, bq_sb, qt_sb, HALO + col0, col0, 512)

            # V: out[row_tile, vcols] = x_T.T @ W_v + bv
            for r in range(NKT):
                for n0 in (0, 384):
                    ps = psA.tile([128, 512], F32, tag="mm")
                    nc.tensor.matmul(
                        ps[:, :384], ones_sb[0:1, :], bv_sb[0:1, n0:n0 + 384],
                        start=True, stop=False,
                    )
                    for c in range(NCT):
                        nc.tensor.matmul(
                            ps[:, :384],
                            xt_sb[:, c, r * 128:(r + 1) * 128],
                            wv_sb[:, c, n0:n0 + 384],
                            start=False,
                            stop=(c == NCT - 1),
                        )
                    h0 = n0 // HD
                    nc.any.tensor_copy(
                        out=v_sb[:, r, h0:h0 + 6, 0:HD],
                        in_=ps[:, :384].rearrange("p (h d) -> p h d", d=HD),
                    )

            # ---- phase 3: attention per head ----
            for h in range(N_HEAD):
                ct = h // 2
                p0 = (h % 2) * HD
                et = et_pool.tile([128, NKT, WIN], BF16, tag="et")
                for kt in range(NKT):
                    ps_s = psA.tile([128, 512], F32, tag="mm")
                    nc.tensor.matmul(
                        ps_s[:, :WIN],
                        kt_sb[p0:p0 + HD, ct, kt * 128:(kt + 1) * 128],
                        qt_sb[p0:p0 + HD, ct, QS[kt]:QS[kt] + WIN],
                        start=True, stop=True,
                    )
                    nc.scalar.activation(
                        et[:, kt, :], ps_s[:, :WIN], AF.Exp, scale=SCALE,
                    )
                nc.vector.tensor_tensor(et[:], et[:], mask_sb[:], OP.mult)

                for half in range(2):
                    ps_y = psY.tile([128, 512], F32, tag="y")
                    qb0 = half * 4
                    mms = []
                    for kt in range(NKT):
                        for qb in (kt - 2, kt - 1, kt):
                            if qb0 <= qb < qb0 + 4:
                                mms.append((kt, qb))
                    for i, (kt, qb) in enumerate(mms):
                        j0 = qb * 128 - QS[kt]
                        nc.tensor.matmul(
                            ps_y[:, (qb - qb0) * 128:(qb - qb0 + 1) * 128],
                            v_sb[:, kt, h, :],
                            et[:, kt, j0:j0 + 128],
                            start=(i == 0),
                            stop=(i == len(mms) - 1),
                            skip_group_check=True,
                        )
                    # 1/D via exp(-ln(D)) on ScalarE: Ln aligned at base 64,
                    # Exp(scale=-1) shift-copies the result down to base 0.
                    rln = wk_pool.tile([128, 512], F32, tag="rln")
                    rec = wk_pool.tile([HD, 512], F32, tag="rec")
                    nc.scalar.activation(rln[HD:128, :], ps_y[HD:128, :], AF.Ln)
                    nc.scalar.activation(rec[:, :], rln[HD:128, :], AF.Exp,
                                         scale=-1.0)
                    nc.vector.tensor_tensor(
                        yn_sb[p0:p0 + HD, ct, half * 512:(half + 1) * 512],
                        ps_y[0:HD, :],
                        rec[:, :],
                        OP.mult,
                    )

            # ---- phase 4: output projection ----
            for qb in range(8):
                o_sb = wk_pool.tile([128, C], F32, tag="osb")
                for n0 in (0, 384):
                    ps = psA.tile([128, 512], F32, tag="mm")
                    nc.tensor.matmul(
                        ps[:, :384], ones_sb[0:1, :], bp_sb[0:1, n0:n0 + 384],
                        start=True, stop=False,
                    )
                    for c in range(NCT):
                        nc.tensor.matmul(
                            ps[:, :384],
                            yn_sb[:, c, qb * 128:(qb + 1) * 128],
                            wp_sb[:, c, n0:n0 + 384],
                            start=False,
                            stop=(c == NCT - 1),
                        )
                    nc.any.tensor_copy(out=o_sb[:, n0:n0 + 384], in_=ps[:, :384])
                nc.sync.dma_start(out[qb * 128:(qb + 1) * 128, :], o_sb[:])

    nc.compile()
    return nc


_NC_CACHE = []


def _get_nc() -> bass.Bass:
    if not _NC_CACHE:
        _NC_CACHE.append(_build_nc())
    return _NC_CACHE[0]


def _make_mask(half: int) -> np.ndarray:
    chunk_start = half * TQ
    p = np.arange(128)[:, None, None]
    kt = np.arange(NKT)[None, :, None]
    j = np.arange(WIN)[None, None, :]
    lk = 128 * kt + p
    qi = np.array(QS)[None, :, None] + j
    band = (qi >= lk - WINDOW) & (qi <= lk - 1)
    exists = (chunk_start - HALO + lk) >= 0
    m = (band & exists).astype(ml_dtypes.bfloat16)
    return m.reshape(128, NKT * WIN)


def build_in_maps(x, W_attn, b_attn, W_proj, b_proj):
    x = np.asarray(x, dtype=np.float32)
    W_attn = np.asarray(W_attn, dtype=np.float32)
    b_attn = np.asarray(b_attn, dtype=np.float32)
    W_proj = np.asarray(W_proj, dtype=np.float32)
    b_proj = np.asarray(b_proj, dtype=np.float32)

    bf = ml_dtypes.bfloat16
    wq_h = W_attn[:, 0:C].astype(bf)
    wk_h = W_attn[:, C:2 * C].astype(bf)
    wv_h = W_attn[:, 2 * C:3 * C].astype(bf)
    wp_h = W_proj.astype(bf)
    bq_h = np.ascontiguousarray(
        b_attn[0:C].reshape(NCT, 128).T).astype(np.float32)
    bk_h = np.ascontiguousarray(
        b_attn[C:2 * C].reshape(NCT, 128).T).astype(np.float32)
    bv_h = b_attn[2 * C:3 * C].reshape(1, C).astype(bf)
    bp_h = b_proj.reshape(1, C).astype(bf)
    masks = [_make_mask(0), _make_mask(1)]

    in_maps = []
    for core in range(8):
        b, half = divmod(core, 2)
        start = half * TQ - HALO
        if start < 0:
            x_win = np.concatenate(
                [np.zeros((HALO, C), np.float32), x[b, 0:TQ]], axis=0)
        else:
            x_win = x[b, start:start + ROWS]
        x_t = np.ascontiguousarray(x_win.T).astype(bf)
        in_maps.append({
            "x_t": x_t, "wq": wq_h, "wk": wk_h, "wv": wv_h, "wp": wp_h,
            "bq": bq_h, "bk": bk_h, "bv": bv_h, "bp": bp_h,
            "mask": masks[half],
        })
    return in_maps


def kernel(x, W_attn, b_attn, W_proj, b_proj):
    in_maps = build_in_maps(x, W_attn, b_attn, W_proj, b_proj)
    nc = _get_nc()
    res = run_bass_kernel_spmd(nc, in_maps, list(range(8)))
    y = np.empty((B, T, C), dtype=np.float32)
    for core in range(8):
        b, half = divmod(core, 2)
        y[b, half * TQ:(half + 1) * TQ, :] = res.results[core]["out"]
    return y


# revision 17
# speedup vs baseline: 1.2634x; 1.2634x over previous
"""Sliding-window causal self-attention (WINDOW=256) on 8 trn2 NeuronCores.

Sharding: 8 cores = 4 batch items x 2 sequence halves (1024 queries each).
Each core receives x pre-transposed with a 256-row key/value halo and
computes its output chunk fully independently (no collectives); the host
concatenates.

On-core dataflow (transpose-free, "key-major" attention):
  x_T [C=768, 1280]  --PE-->  Q_T [768, 1024], K_T [768, 1280]  (head dim on
  partitions), V [1280, 768] (+ a 64-wide ones block per head for softmax
  denominators).
  Per head, per 128-key tile: S_T [128k, 384q] = K_T.T @ Q_T (K=64 matmul),
  E = exp(S/8) via ScalarE (scale fused), band+validity mask multiply (DVE),
  then AV matmuls accumulate Y (rows 0-63) and the replicated denominator
  (rows 64-127) into PSUM per 128-query block.  Normalize with
  reciprocal_approx_fast, project with W_proj (bias via K=1 ones-row matmul).
"""

import numpy as np
import ml_dtypes

import concourse.bass as bass
import concourse.bacc as bacc
import concourse.mybir as mybir
from concourse.tile import TileContext
from concourse.bass_utils import run_bass_kernel_spmd

F32 = mybir.dt.float32
BF16 = mybir.dt.bfloat16
AF = mybir.ActivationFunctionType
OP = mybir.AluOpType

N_HEAD = 12
WINDOW = 256
B, T, C = 4, 2048, 768
HD = C // N_HEAD              # 64
TQ = 1024                     # queries per core
HALO = 256
ROWS = TQ + HALO              # 1280 rows of k/v per core
NCT = C // 128                # 6 contraction tiles
NKT = ROWS // 128             # 10 key tiles
WIN = 384                     # q-window width per key tile
SCALE = 1.0 / float(np.sqrt(HD))

# q-window start per key tile (compile-time, same on every core)
QS = [min(max(128 * (kt - 2), 0), TQ - WIN) for kt in range(NKT)]


def _build_nc() -> bass.Bass:
    nc = bacc.Bacc()

    x_t = nc.dram_tensor("x_t", [C, ROWS], BF16, kind="ExternalInput")
    wq = nc.dram_tensor("wq", [C, C], BF16, kind="ExternalInput")
    wk = nc.dram_tensor("wk", [C, C], BF16, kind="ExternalInput")
    wv = nc.dram_tensor("wv", [C, C], BF16, kind="ExternalInput")
    wp = nc.dram_tensor("wp", [C, C], BF16, kind="ExternalInput")
    bq = nc.dram_tensor("bq", [1, C], BF16, kind="ExternalInput")
    bk = nc.dram_tensor("bk", [1, C], BF16, kind="ExternalInput")
    bv = nc.dram_tensor("bv", [1, C], BF16, kind="ExternalInput")
    bp = nc.dram_tensor("bp", [1, C], BF16, kind="ExternalInput")
    mask = nc.dram_tensor("mask", [128, NKT * WIN], BF16, kind="ExternalInput")
    out = nc.dram_tensor("out", [TQ, C], F32, kind="ExternalOutput")

    with TileContext(nc) as tc:
        with (
            tc.tile_pool(name="persist", bufs=1) as pp,
            tc.tile_pool(name="work", bufs=3) as wk_pool,
            tc.tile_pool(name="et", bufs=2) as et_pool,
            tc.tile_pool(name="psA", bufs=2, space="PSUM") as psA,
            tc.tile_pool(name="psS", bufs=2, space="PSUM") as psS,
            tc.tile_pool(name="psY", bufs=2, space="PSUM") as psY,
        ):
            # ---- persistent SBUF tensors ----
            xt_sb = pp.tile([128, NCT, ROWS], BF16)
            wq_sb = pp.tile([128, NCT, C], BF16)
            wk_sb = pp.tile([128, NCT, C], BF16)
            wv_sb = pp.tile([128, NCT, C], BF16)
            wp_sb = pp.tile([128, NCT, C], BF16)
            bq_sb = pp.tile([1, C], BF16)
            bk_sb = pp.tile([1, C], BF16)
            bv_sb = pp.tile([1, C], BF16)
            bp_sb = pp.tile([1, C], BF16)
            mask_sb = pp.tile([128, NKT, WIN], BF16)
            ones_sb = pp.tile([1, 512], BF16)

            qt_sb = pp.tile([128, NCT, TQ], BF16)     # Q_T: head dims on partitions
            kt_sb = pp.tile([128, NCT, ROWS], BF16)   # K_T
            v_sb = pp.tile([128, NKT, N_HEAD, 128], BF16)  # [V(64) | ones(64)]
            yn_sb = pp.tile([128, NCT, TQ], BF16)     # normalized Y_T

            nc.sync.dma_start(xt_sb[:], x_t.rearrange("(t p) n -> p t n", p=128))
            nc.sync.dma_start(wk_sb[:], wk.rearrange("(t p) n -> p t n", p=128))
            nc.sync.dma_start(wq_sb[:], wq.rearrange("(t p) n -> p t n", p=128))
            nc.sync.dma_start(wv_sb[:], wv.rearrange("(t p) n -> p t n", p=128))
            nc.sync.dma_start(wp_sb[:], wp.rearrange("(t p) n -> p t n", p=128))
            nc.sync.dma_start(bq_sb[:], bq[:])
            nc.sync.dma_start(bk_sb[:], bk[:])
            nc.sync.dma_start(bv_sb[:], bv[:])
            nc.sync.dma_start(bp_sb[:], bp[:])
            nc.sync.dma_start(mask_sb[:], mask.rearrange("p (k w) -> p k w", w=WIN))
            nc.vector.memset(ones_sb[:], 1.0)
            nc.vector.memset(v_sb[:, :, :, HD:128], 1.0)

            # ---- phase 2: projections ----
            # K_T / Q_T: out[outdim_tile, seq] = W.T @ x_T
            def proj_T(w_sb, b_sb, dst, xcol0, dcol0, ncols):
                for m in range(NCT):
                    ps = psA.tile([128, 512], F32, tag="mm")
                    # bias along outdim (partitions) as a K=1 rank-1 matmul
                    nc.tensor.matmul(
                        ps[:, :ncols],
                        b_sb[0:1, m * 128:(m + 1) * 128],
                        ones_sb[0:1, :ncols],
                        start=True, stop=False,
                    )
                    for c in range(NCT):
                        nc.tensor.matmul(
                            ps[:, :ncols],
                            w_sb[:, c, m * 128:(m + 1) * 128],
                            xt_sb[:, c, xcol0:xcol0 + ncols],
                            start=False,
                            stop=(c == NCT - 1),
                        )
                    nc.any.tensor_copy(
                        out=dst[:, m, dcol0:dcol0 + ncols], in_=ps[:, :ncols],
                    )

            for col0, ncols in ((0, 512), (512, 512), (1024, 256)):
                proj_T(wk_sb, bk_sb, kt_sb, col0, col0, ncols)
            for col0 in (0, 512):
                proj_T(wq_sb, bq_sb, qt_sb, HALO + col0, col0, 512)

            # V: out[row_tile, vcols] = x_T.T @ W_v + bv
            for r in range(NKT):
                for n0 in (0, 384):
                    ps = psA.tile([128, 512], F32, tag="mm")
                    nc.tensor.matmul(
                        ps[:, :384], ones_sb[0:1, :128], bv_sb[0:1, n0:n0 + 384],
                        start=True, stop=False,
                    )
                    for c in range(NCT):
                        nc.tensor.matmul(
                            ps[:, :384],
                            xt_sb[:, c, r * 128:(r + 1) * 128],
                            wv_sb[:, c, n0:n0 + 384],
                            start=False,
                            stop=(c == NCT - 1),
                        )
                    h0 = n0 // HD
                    nc.any.tensor_copy(
                        out=v_sb[:, r, h0:h0 + 6, 0:HD],
                        in_=ps[:, :384].rearrange("p (h d) -> p h d", d=HD),
                    )

            # ---- phase 3: attention per head ----
            for h in range(N_HEAD):
                ct = h // 2
                p0 = (h % 2) * HD
                et = et_pool.tile([128, NKT, WIN], BF16, tag="et")
                for kt2 in range(0, NKT, 2):
                    ps_s = psS.tile([128, 2, 512], F32, tag="ss")
                    for j in range(2):
                        kt = kt2 + j
                        nc.tensor.matmul(
                            ps_s[:, j, :WIN],
                            kt_sb[p0:p0 + HD, ct, kt * 128:(kt + 1) * 128],
                            qt_sb[p0:p0 + HD, ct, QS[kt]:QS[kt] + WIN],
                            start=True, stop=True,
                        )
                    nc.scalar.activation(
                        et[:, kt2:kt2 + 2, :], ps_s[:, :, :WIN], AF.Exp,
                        scale=SCALE,
                    )
                nc.vector.tensor_tensor(et[:], et[:], mask_sb[:], OP.mult)

                for half in range(2):
                    ps_y = psY.tile([128, 512], F32, tag="y")
                    qb0 = half * 4
                    mms = []
                    for kt in range(NKT):
                        for qb in (kt - 2, kt - 1, kt):
                            if qb0 <= qb < qb0 + 4:
                                mms.append((kt, qb))
                    for i, (kt, qb) in enumerate(mms):
                        j0 = qb * 128 - QS[kt]
                        nc.tensor.matmul(
                            ps_y[:, (qb - qb0) * 128:(qb - qb0 + 1) * 128],
                            v_sb[:, kt, h, :],
                            et[:, kt, j0:j0 + 128],
                            start=(i == 0),
                            stop=(i == len(mms) - 1),
                            skip_group_check=True,
                        )
                    # 1/D via exp(-ln(D)) on ScalarE: Ln aligned at base 64,
                    # Exp(scale=-1) shift-copies the result down to base 0.
                    rln = wk_pool.tile([128, 512], F32, tag="rln")
                    rec = wk_pool.tile([HD, 512], F32, tag="rec")
                    nc.scalar.activation(rln[HD:128, :], ps_y[HD:128, :], AF.Ln)
                    nc.scalar.activation(rec[:, :], rln[HD:128, :], AF.Exp,
                                         scale=-1.0)
                    nc.vector.tensor_tensor(
                        yn_sb[p0:p0 + HD, ct, half * 512:(half + 1) * 512],
                        ps_y[0:HD, :],
                        rec[:, :],
                        OP.mult,
                    )

            # ---- phase 4: output projection ----
            for qb in range(8):
                o_sb = wk_pool.tile([128, C], F32, tag="osb")
                for n0 in (0, 384):
                    ps = psA.tile([128, 512], F32, tag="mm")
                    nc.tensor.matmul(
                        ps[:, :384], ones_sb[0:1, :128], bp_sb[0:1, n0:n0 + 384],
                        start=True, stop=False,
                    )
                    for c in range(NCT):
                        nc.tensor.matmul(
                            ps[:, :384],
                            yn_sb[:, c, qb * 128:(qb + 1) * 128],
                            wp_sb[:, c, n0:n0 + 384],
                            start=False,
                            stop=(c == NCT - 1),
                        )
                    nc.any.tensor_copy(out=o_sb[:, n0:n0 + 384], in_=ps[:, :384])
                nc.sync.dma_start(out[qb * 128:(qb + 1) * 128, :], o_sb[:])

    nc.compile()
    return nc


_NC_CACHE = []


def _get_nc() -> bass.Bass:
    if not _NC_CACHE:
        _NC_CACHE.append(_build_nc())
    return _NC_CACHE[0]


def _make_mask(half: int) -> np.ndarray:
    chunk_start = half * TQ
    p = np.arange(128)[:, None, None]
    kt = np.arange(NKT)[None, :, None]
    j = np.arange(WIN)[None, None, :]
    lk = 128 * kt + p
    qi = np.array(QS)[None, :, None] + j
    band = (qi >= lk - WINDOW) & (qi <= lk - 1)
    exists = (chunk_start - HALO + lk) >= 0
    m = (band & exists).astype(ml_dtypes.bfloat16)
    return m.reshape(128, NKT * WIN)


def build_in_maps(x, W_attn, b_attn, W_proj, b_proj):
    x = np.asarray(x, dtype=np.float32)
    W_attn = np.asarray(W_attn, dtype=np.float32)
    b_attn = np.asarray(b_attn, dtype=np.float32)
    W_proj = np.asarray(W_proj, dtype=np.float32)
    b_proj = np.asarray(b_proj, dtype=np.float32)

    bf = ml_dtypes.bfloat16
    wq_h = W_attn[:, 0:C].astype(bf)
    wk_h = W_attn[:, C:2 * C].astype(bf)
    wv_h = W_attn[:, 2 * C:3 * C].astype(bf)
    wp_h = W_proj.astype(bf)
    bq_h = b_attn[0:C].reshape(1, C).astype(bf)
    bk_h = b_attn[C:2 * C].reshape(1, C).astype(bf)
    bv_h = b_attn[2 * C:3 * C].reshape(1, C).astype(bf)
    bp_h = b_proj.reshape(1, C).astype(bf)
    masks = [_make_mask(0), _make_mask(1)]

    in_maps = []
    for core in range(8):
        b, half = divmod(core, 2)
        start = half * TQ - HALO
        if start < 0:
            x_win = np.concatenate(
                [np.zeros((HALO, C), np.float32), x[b, 0:TQ]], axis=0)
        else:
            x_win = x[b, start:start + ROWS]
        x_t = np.ascontiguousarray(x_win.T).astype(bf)
        in_maps.append({
            "x_t": x_t, "wq": wq_h, "wk": wk_h, "wv": wv_h, "wp": wp_h,
            "bq": bq_h, "bk": bk_h, "bv": bv_h, "bp": bp_h,
            "mask": masks[half],
        })
    return in_maps


def kernel(x, W_attn, b_attn, W_proj, b_proj):
    in_maps = build_in_maps(x, W_attn, b_attn, W_proj, b_proj)
    nc = _get_nc()
    res = run_bass_kernel_spmd(nc, in_maps, list(range(8)))
    y = np.empty((B, T, C), dtype=np.float32)
    for core in range(8):
        b, half = divmod(core, 2)
        y[b, half * TQ:(half + 1) * TQ, :] = res.results[core]["out"]
    return y
